# revision 81
# baseline (speedup 1.0000x reference)
"""DeformTransformerBlock2D Trainium2 kernel (8-core SPMD, full I/O).

Sharding: core k handles batch k//4, image rows [20*(k%4), 20*(k%4)+20)
(3200 output positions). Each core computes the full-image value projection
for its batch (anchors are uniform-random, so the bilinear gather is global).

Bilinear gather: 4x4 round-centered window per position (clips only when
|offset| > 1 px; host-checked rel err 4.7e-3 vs the reference). One SWDGE
dma_gather per 128-position chunk fetches 8 dx-pair rows x 512B fp8 per
position: partition (s, dy, dxp) = s*8 + dy*2 + dxp, s in [0,16) within a
16-position block, 8 blocks per chunk -> exactly 128 partitions.

Weight pipeline (DVE fast modes): hm = min(|u-j|-1, 0) = -hat via
tensor_scalar (4x); tmp = hm_x*(hm_y*attn) via 2x tensor_tensor (signs
cancel); p-tree-reduce; per-(g,eo) PE transpose with an s-broadcast
stationary replicates the 8 window-rows into all 16 s-blocks; the
block-diag s-mask rides the single psum->SBUF staging multiply.

LN stats: mean/E[z^2] via PE ones-matmuls, variance folded in psum with a
-1 K=1 matmul of mean^2, rstd = exp(-0.5*ln(var+eps)) on Act (keeps the
whole kernel in the natural_log_exp act table except SiLU), stats
replicated across partitions by gpsimd.partition_broadcast so the
normalize runs at DVE 2x. Three act-table epochs total.
"""

import os
import numpy as np
import ml_dtypes

import concourse.bacc as bacc
import concourse.bass as bass
import concourse.tile as tile
from concourse import mybir
from concourse.bass_utils import run_bass_kernel_spmd

F32 = mybir.dt.float32
F16 = mybir.dt.float16
BF16 = mybir.dt.bfloat16
FP8 = mybir.dt.float8e4
I16 = mybir.dt.int16
AX = mybir.AxisListType
ALU = mybir.AluOpType
ACTF = mybir.ActivationFunctionType

B, C, H, W = 2, 256, 80, 160
G, P_PTS = 8, 8
HW = H * W                     # 12800
NCORES = 8
NLOC = 3200                    # positions per core
NCH = 25                       # chunks of 128 positions
WX, WY = 4, 4                  # gather window: 4 wide (2 dx-pairs), 4 tall
NDXP = WX // 2                 # dx pairs per row (512B gather elems)
NROW = WY * NDXP               # 8 (dy, dxp) rows per position
SBLK = 16                      # positions per matmul block
NBLK = 8                       # blocks per 128-chunk
LN_EPS = 1e-5

_CACHE = {}


def _nsplit(total, step):
    o, out = 0, []
    while o < total:
        out.append((o, min(step, total - o)))
        o += step
    return out


def _build_program():
    nc = bacc.Bacc("TRN2", target_bir_lowering=False, debug=False,
                   num_devices=NCORES)

    d = {}
    def din(name, shape, dt):
        d[name] = nc.dram_tensor(name, shape, dt, kind="ExternalInput")
    din("qsum8", (2, 128, HW), FP8)
    din("qloc8", (2, 128, NLOC), FP8)
    din("qbin", (2, 128, NLOC), BF16)     # residual, outb pre-folded (host)
    din("axy2", (128, NCH * 2), F32)
    din("j68", (128, WX * P_PTS), F16)    # j grid replicated over p
    din("gidx", (128, NCH * 64), I16)
    din("vW", (256, 256), BF16)
    din("oaW", (256, 192), BF16)
    din("oabrow", (1, 192), BF16)         # off/attn bias row (K=1 matmul)
    din("outW", (256, 256), BF16)
    din("w1T", (256, 512), BF16)
    din("b1", (128, 4), F32)
    din("w2T", (512, 256), BF16)
    din("b2", (128, 2), F32)
    din("ln1g", (128, 2), F32)
    din("ln1b", (128, 2), F32)
    din("ln2g", (128, 2), F32)
    din("ln2b", (128, 2), F32)
    din("identh", (128, 128), F16)
    din("maskW", (128, SBLK * WY * WX), F16)  # [n, s, (dy,dx)] = (n%16==s)
    din("ones", (128, 1), BF16)           # column of 1/256 (mean matmul)
    din("ones128b", (1, 128), BF16)       # row of ones (K=1 bias matmul)
    din("negone", (1, 1), BF16)           # -1 (variance fold)
    din("epsc", (1, 1), F32)              # LN epsilon (Act bias)
    din("nhalf", (1, 1), F32)             # -0.5 (Act scale for rstd)

    d["y_out"] = nc.dram_tensor("y_out", (2, 128, NLOC), F32,
                                kind="ExternalOutput")
    d["v8"] = nc.dram_tensor("v8scratch", (HW, 256), FP8)

    with tile.TileContext(nc) as tc:
        _emit(nc, tc, d)
    nc.compile()
    return nc


def _ld(nc, pool, dram, shape, dt, rearr=None, **rkw):
    t = pool.tile(shape, dt, tag="ld_" + dram.name)
    src = dram.ap()
    if rearr:
        src = src.rearrange(rearr, **rkw)
    nc.sync.dma_start(out=t, in_=src)
    return t


def _emit(nc, tc, d):
    import os as _os
    ABL = set(_os.environ.get("KABL", "").split(","))
    from contextlib import ExitStack
    ctx = ExitStack()
    pconst = ctx.enter_context(tc.tile_pool(name="pconst", bufs=1))
    pmain = ctx.enter_context(tc.tile_pool(name="pmain", bufs=1))
    ppsA = ctx.enter_context(tc.tile_pool(name="ppsA", bufs=2, space="PSUM"))
    ppsT = ctx.enter_context(tc.tile_pool(name="ppsT", bufs=1, space="PSUM"))

    # ---------- phase-1-critical input loads first ----------
    pin = ctx.enter_context(tc.tile_pool(name="pin", bufs=1))
    ql8 = _ld(nc, pin, d["qloc8"], [128, 2, NLOC], FP8, "kt k n -> k kt n")
    # ---------- constants ----------
    oaW = _ld(nc, pconst, d["oaW"], [128, 2, 192], BF16, "(kt k) m -> k kt m", k=128)
    oabrow = _ld(nc, pconst, d["oabrow"], [1, 192], BF16)
    ones128b = _ld(nc, pconst, d["ones128b"], [1, 128], BF16)
    vW = _ld(nc, pconst, d["vW"], [128, 2, 256], BF16, "(kt k) m -> k kt m", k=128)
    axy2 = _ld(nc, pconst, d["axy2"], [128, NCH, 2], F32)
    j68 = _ld(nc, pconst, d["j68"], [128, WX, P_PTS], F16)
    outW = _ld(nc, pconst, d["outW"], [128, 2, 256], BF16, "(kt k) m -> k kt m", k=128)
    w1T = _ld(nc, pconst, d["w1T"], [128, 2, 512], BF16, "(kt k) m -> k kt m", k=128)
    b1 = _ld(nc, pconst, d["b1"], [128, 4], F32)
    w2T = _ld(nc, pconst, d["w2T"], [128, 4, 256], BF16, "(kt k) m -> k kt m", k=128)
    b2 = _ld(nc, pconst, d["b2"], [128, 2], F32)
    ln1g = _ld(nc, pconst, d["ln1g"], [128, 2], F32)
    ln1b = _ld(nc, pconst, d["ln1b"], [128, 2], F32)
    ln2g = _ld(nc, pconst, d["ln2g"], [128, 2], F32)
    ln2b = _ld(nc, pconst, d["ln2b"], [128, 2], F32)
    gidx = _ld(nc, pconst, d["gidx"], [128, NCH * 64], I16)
    identh = _ld(nc, pconst, d["identh"], [128, 128], F16)
    maskW = _ld(nc, pconst, d["maskW"], [128, SBLK, WY * WX], F16)
    ones = _ld(nc, pconst, d["ones"], [128, 1], BF16)
    negone = _ld(nc, pconst, d["negone"], [1, 1], BF16)
    epsc = _ld(nc, pconst, d["epsc"], [1, 1], F32)
    nhalf = _ld(nc, pconst, d["nhalf"], [1, 1], F32)

    # ---------- persistent activations ----------
    qb = pmain.tile([128, 2, NLOC], BF16)
    aggT = pmain.tile([128, 2, NLOC], BF16)
    y1b = pmain.tile([128, 2, NLOC], BF16)
    hbF = pmain.tile([128, 4, NLOC], BF16)

    pmid_cm = tc.tile_pool(name="pmid", bufs=1)
    pmid = pmid_cm.__enter__()
    offa = pmid.tile([128, NCH, 192], F16)
    attnN = pmid.tile([128, NCH, G, P_PTS], F16)

    # ========== phase 1: value field + projections ==========
    with tc.tile_pool(name="ph1", bufs=1) as p1, \
         tc.tile_pool(name="ph1t", bufs=4) as p1t, \
         tc.tile_pool(name="ppsB", bufs=2, space="PSUM") as ppsB:
        nc.sync.dma_start(out=qb, in_=d["qbin"].ap()
                          .rearrange("kt k n -> k kt n"))
        qiap = d["qsum8"].ap().rearrange("kt k n -> k kt n")

        # off/attn projections, bias folded in via a K=1 matmul
        for c in range(NCH):
            ps = ppsB.tile([128, 192], F32, tag="psB")
            for kt in range(2):
                nc.tensor.matmul(ps, ql8[:, kt, c * 128:(c + 1) * 128],
                                 oaW[:, kt, :], start=(kt == 0), stop=False)
            nc.tensor.matmul(ps, ones128b, oabrow, start=False, stop=True)
            if c % 2 == 0:
                nc.vector.tensor_copy(out=offa[:, c], in_=ps)
            else:
                nc.scalar.activation(offa[:, c], ps, ACTF.Copy)
            # softmax over points
            ae = p1t.tile([128, G, P_PTS], F32, tag="ae")
            nc.scalar.activation(ae.rearrange("n g p -> n (g p)"),
                                 offa[:, c, 128:192], ACTF.Exp)
            ssum = p1t.tile([128, G], F32, tag="ssum")
            nc.vector.tensor_reduce(ssum, ae, axis=AX.X, op=ALU.add)
            srec = p1t.tile([128, G], F32, tag="srec")
            nc.vector.reciprocal(srec, ssum)
            nc.vector.tensor_mul(attnN[:, c], ae,
                                 srec.unsqueeze(2).broadcast_to([128, G, P_PTS]))

        # value projection, position-major output: lhsT = q chunk (fp8
        # stationary [128ch, 128pos]), rhs = vW row block -> psum[pos, 256ch]
        for pc in range(25 if "noph1v" not in ABL else 0):  # 512-px chunks
            no = pc * 512
            qc = p1t.tile([128, 2, 512], FP8, tag="qc")
            nc.sync.dma_start(out=qc, in_=qiap[:, :, no:no + 512])
            vrow = p1t.tile([128, 2, 2, 256], FP8, tag="vrow")
            for half in range(2):
                ps = ppsB.tile([128, 2, 256], F32, tag="psPM")
                for b in range(2):
                    blk = half * 2 + b
                    for kt in range(2):
                        nc.tensor.matmul(
                            ps[:, b], qc[:, kt, blk * 128:(blk + 1) * 128],
                            vW[:, kt, :], start=(kt == 0), stop=(kt == 1))
                nc.scalar.activation(
                    vrow[:, half].rearrange("n b c -> n (b c)"),
                    ps.rearrange("n b c -> n (b c)"), ACTF.Copy)
            v8out = bass.AP(tensor=d["v8"], offset=no * 256,
                            ap=[[256, 128], [128 * 256, 4], [1, 256]])
            nc.sync.dma_start(out=v8out, in_=vrow.rearrange(
                "n h b c -> n (h b) c"))

    # ========== phase 2 + sweep A (out-proj + LN1), interleaved ==========
    v8in = bass.AP(tensor=d["v8"], offset=0,
               ap=[[256, HW - 1], [1, 512]])
    with tc.tile_pool(name="ph2w", bufs=4) as p2w, \
         tc.tile_pool(name="ph2m", bufs=3) as p2m, \
         tc.tile_pool(name="ph2t", bufs=3) as p2t, \
         tc.tile_pool(name="ph3t", bufs=3) as p3t, \
         tc.tile_pool(name="ppsW", bufs=2, space="PSUM") as ppsW, \
         tc.tile_pool(name="ppsM", bufs=1, space="PSUM") as ppsM:
        done_tiles = []
        done_b = []
        done_c = []
        def flush_tiles(upto):
            for no, nn in _nsplit(NLOC, 512):
                if no + nn <= upto and (no, nn) not in done_tiles:
                    done_tiles.append((no, nn))
                    if "noph3" not in ABL:
                        _sweep_a_tile(nc, ppsA, ppsM, p3t, qb, aggT,
                                      y1b, outW, ln1g, ln1b, ones, negone,
                                      epsc, nhalf, no, nn)
            # once the first 4 tiles' LN1 is out, start their FFN1+SiLU
            # batch (one extra act-table round-trip, fills the B bubble)
            if len(done_tiles) >= 4 and not done_b and "noph3" not in ABL:
                for no, nn in _nsplit(NLOC, 512)[:4]:
                    done_b.append((no, nn))
                    _sweep_b_tile(nc, ppsA, p3t, y1b, hbF, w1T, b1, no, nn)
                for no, nn in _nsplit(NLOC, 512)[:3]:
                    done_c.append((no, nn))
                    _sweep_c_tile(nc, d, ppsA, ppsM, p3t, y1b, hbF, w2T,
                                  b2, ln2g, ln2b, ones, negone, epsc, nhalf,
                                  no, nn)
        wins = {}

        def issue_gather(c):
            # windows: partition (s, dy, dxp) holds, per 16-pos block, the
            # 512B dx-pair (2 taps x 256ch fp8) of position blk*16+s.
            win = p2w.tile([128, NBLK, 512], FP8, tag="win")
            nc.gpsimd.dma_gather(
                out_ap=win, in_ap=v8in,
                idxs_ap=gidx[:, c * 64:(c + 1) * 64],
                num_idxs=1024, num_idxs_reg=1024,
                elem_size=512, elem_step=256)
            wins[c] = win

        if "nogather" not in ABL:
            issue_gather(0)
            issue_gather(1)
            issue_gather(2)
        for c in range(NCH):
            flush_tiles(c * 128)
            if "nogather" in ABL:
                continue
            if c + 3 < NCH:
                issue_gather(c + 3)
            win = wins.pop(c)

            if "nowt" in ABL:
                continue
            # u = off + anchor (in-window coords), packed f16
            u = p2t.tile([128, 2, G, P_PTS], F16, tag="u")
            offc = offa[:, c, 0:128].rearrange(
                "n (g p two) -> n two g p", g=G, two=2)
            for xy in range(2):
                nc.vector.tensor_scalar(u[:, xy], offc[:, xy],
                                        axy2[:, c, xy:xy + 1], None,
                                        op0=ALU.add)
            # d = u - j over the 4-tap grid; hm = min(|d|-1, 0) = -hat
            dt_ = p2t.tile([128, 2 * G, WX, P_PTS], F16, tag="dt")
            nc.vector.tensor_tensor(
                dt_, u.rearrange("n two g p -> n (two g) p").unsqueeze(2)
                .broadcast_to([128, 2 * G, WX, P_PTS]),
                j68.unsqueeze(1).broadcast_to([128, 2 * G, WX, P_PTS]),
                op=ALU.subtract)
            dtf = dt_.rearrange("n ag j p -> n (ag j p)")
            t1 = p2t.tile([128, 2 * G * WX * P_PTS], F16, tag="t1")
            nc.vector.tensor_scalar(t1, dtf, -1.0, 2.0, op0=ALU.mult,
                                    op1=ALU.add)
            hm = p2t.tile([128, 2, G, WX, P_PTS], F16, tag="hm")
            hmf = hm.rearrange("n a g j p -> n (a g j p)")
            nc.vector.tensor_tensor(hmf, t1, dtf, op=ALU.min)
            nc.vector.tensor_scalar(hmf, hmf, 0.0, None, op0=ALU.max)
            # cy2 = hm_y * attn  (= -hat_y*attn, <= 0)
            cy2 = p2t.tile([128, G, WY, P_PTS], F16, tag="cy2")
            nc.vector.tensor_mul(
                cy2, hm[:, 1, :, 0:WY, :], attnN[:, c].unsqueeze(2)
                .broadcast_to([128, G, WY, P_PTS]))
            # tmp[g,dy,dx,p] = cy2[g,dy,p] * hm_x[g,dx,p]  (signs cancel)
            tmp = p2m.tile([128, G, WY, WX, P_PTS], F16, tag="fmatmp")
            nc.vector.tensor_mul(
                tmp, cy2.unsqueeze(3).broadcast_to([128, G, WY, WX, P_PTS]),
                hm[:, 0].unsqueeze(2).broadcast_to([128, G, WY, WX, P_PTS]))
            tf = tmp.rearrange("n g dy dx p -> n (g dy dx) p")
            nc.vector.tensor_add(tf[:, :, 0:4], tf[:, :, 0:4], tf[:, :, 4:8])
            nc.vector.tensor_add(tf[:, :, 0:2], tf[:, :, 0:2], tf[:, :, 2:4])
            cwb = p2m.tile([128, G, WY, WX], F16, tag="cwb")
            nc.vector.tensor_add(cwb.rearrange("n g dy dx -> n (g dy dx)"),
                                 tf[:, :, 0], tf[:, :, 1])
            # block-diag s-mask expansion in n-major (wide mask keeps 2x)
            cwm = p2m.tile([128, G, SBLK, WY * WX], F16, tag="cwm")
            cwbf = cwb.rearrange("n g dy dx -> n g (dy dx)")
            nc.vector.tensor_mul(
                cwm, cwbf.unsqueeze(2).broadcast_to([128, G, SBLK, WY * WX]),
                maskW.unsqueeze(1).broadcast_to([128, G, SBLK, WY * WX]))
            # transpose per (g, eo): pst[(s,dy,dxp), 2g+eo, n]
            pst = ppsW.tile([128, 2 * G, 128], F16, tag="cwT")
            cwmv = cwm.rearrange("n g s (dy dxp eo) -> n g eo s dy dxp",
                                 eo=2, dy=WY)
            for g in range(G):
                for eo in range(2):
                    nc.tensor.transpose(pst[:, 2 * g + eo, :],
                                        cwmv[:, g, eo], identh)
            # psum -> SBUF rhs staging, split across Act/DVE by parity
            cwsb = p2w.tile([128, 2 * G, 128], F16, tag="cwsb")
            if c % 3 == 2 or c >= 23:
                nc.vector.tensor_copy(out=cwsb, in_=pst)
            else:
                nc.scalar.copy(cwsb, pst)
            if "nofma" in ABL:
                continue
            # window contraction on the PE: windows stationary, weights moving
            psa = ppsT.tile([128, 2, 128], F32, tag="psagg")
            for blk in range(NBLK):
                for g in range(G):
                    kt, gq = g // 4, g % 4
                    for eo in range(2):
                        nc.tensor.matmul(
                            psa[gq * 32:(gq + 1) * 32, kt,
                                blk * SBLK:(blk + 1) * SBLK],
                            win[:, blk, eo * 256 + g * 32:
                                eo * 256 + (g + 1) * 32],
                            cwsb[:, 2 * g + eo, blk * SBLK:(blk + 1) * SBLK],
                            start=(eo == 0), stop=(eo == 1),
                            tile_position=(0, gq * 32))
            for kt in range(2):
                if c >= 23:
                    nc.vector.tensor_copy(
                        out=aggT[:, kt, c * 128:(c + 1) * 128],
                        in_=psa[:, kt, :])
                else:
                    nc.scalar.activation(aggT[:, kt, c * 128:(c + 1) * 128],
                                         psa[:, kt, :], ACTF.Copy)
        flush_tiles(NLOC)

        # ===== sweep B: FFN1 + SiLU (one act-table epoch) =====
        if "noph3" not in ABL:
            for no, nn in _nsplit(NLOC, 512):
                if (no, nn) not in done_b:
                    _sweep_b_tile(nc, ppsA, p3t, y1b, hbF, w1T, b1, no, nn)
            # ===== sweep C: FFN2 + LN2 + output =====
            for no, nn in _nsplit(NLOC, 512):
                if (no, nn) not in done_c:
                    _sweep_c_tile(nc, d, ppsA, ppsM, p3t, y1b, hbF, w2T,
                                  b2, ln2g, ln2b, ones, negone, epsc, nhalf,
                                  no, nn)

    pmid_cm.__exit__(None, None, None)

    if "noph3" in ABL:
        for kt in range(2):
            nc.sync.dma_start(out=d["y_out"][kt], in_=qb[:, kt])
    ctx.close()


def _ln_norm(nc, ppsM, p3t, zt, ones, negone, epsc, nhalf, nn,
             tail=False):
    """LN stats for zt [128, 2, nn] bf16 -> rep [128, 2, nn] f16 SBUF
    (partition-broadcast rows: [:,0]=rstd, [:,1]=mean)."""
    psmv = ppsM.tile([128, 512], F32, tag="psM")
    psm = psmv[0:1]
    psv = psmv[32:33]
    for kt in range(2):
        nc.tensor.matmul(psm[:1, :nn], ones, zt[:, kt, :nn],
                         start=(kt == 0), stop=(kt == 1),
                         tile_position=(0, 0))
    sqt = p3t.tile([128, 2, 512], BF16, tag="lnsq")
    eng = nc.vector if tail else nc.gpsimd
    eng.tensor_mul(sqt[:, :, :nn], zt[:, :, :nn], zt[:, :, :nn])
    m2b = p3t.tile([1, 512], BF16, tag="m2b")
    nc.scalar.activation(m2b[:1, :nn], psm[:1, :nn], ACTF.Square)
    for kt in range(2):
        nc.tensor.matmul(psv[:1, :nn], ones, sqt[:, kt, :nn],
                         start=(kt == 0), stop=False,
                         tile_position=(0, 32))
    nc.tensor.matmul(psv[:1, :nn], negone, m2b[:1, :nn],
                     start=False, stop=True, tile_position=(0, 32))
    sq = p3t.tile([1, 512], F32, tag="sq")
    nc.scalar.activation(sq[:1, :nn], psv[:1, :nn], ACTF.Sqrt, bias=epsc)
    stat = p3t.tile([1, 2, 512], F16, tag="stat")
    with nc.allow_low_precision(reason="LN stats fit f16"):
        nc.vector.reciprocal(stat[:1, 0, :nn], sq[:1, :nn])
        if tail:
            nc.vector.tensor_copy(out=stat[:1, 1, :nn], in_=psm[:1, :nn])
        else:
            nc.scalar.activation(stat[:1, 1, :nn], psm[:1, :nn], ACTF.Copy)
    rep = p3t.tile([128, 2, 512], F16, tag="rep")
    if nn == 512:
        nc.gpsimd.partition_broadcast(rep, stat[0:1])
    else:
        for j in range(2):
            nc.gpsimd.partition_broadcast(rep[:, j, :nn], stat[0:1, j, :nn])
    return rep


def _norm_apply(nc, p3t, zt, rep, nn):
    nrm = p3t.tile([128, 2, 512], F16, tag="nrm3")
    with nc.allow_low_precision(reason="LN normalize f16"):
        nc.vector.tensor_sub(
            nrm[:, :, :nn], zt[:, :, :nn],
            rep[:, 1:2, :nn].broadcast_to([128, 2, nn]))
        nc.vector.tensor_mul(
            nrm[:, :, :nn], nrm[:, :, :nn],
            rep[:, 0:1, :nn].broadcast_to([128, 2, nn]))
    return nrm


def _sweep_a_tile(nc, ppsA, ppsM, p3t, qb, aggT, y1b, outW,
                  ln1g, ln1b, ones, negone, epsc, nhalf, no, nn):
    """out-proj + residual + LN1 -> y1b (bf16) for positions [no, no+nn)."""
    zt = p3t.tile([128, 2, 512], BF16, tag="zt3")
    for mt in range(2):
        ps = ppsA.tile([128, 512], F32, tag="psA")
        for kt in range(2):
            nc.tensor.matmul(ps[:, :nn], outW[:, kt, mt * 128:(mt + 1) * 128],
                             aggT[:, kt, no:no + nn],
                             start=(kt == 0), stop=(kt == 1))
        with nc.allow_low_precision(reason="LN input bf16 ok"):
            nc.vector.tensor_add(zt[:, mt, :nn], ps[:, :nn],
                                 qb[:, mt, no:no + nn])
    rep = _ln_norm(nc, ppsM, p3t, zt, ones, negone, epsc, nhalf, nn)
    nrm = _norm_apply(nc, p3t, zt, rep, nn)
    with nc.allow_low_precision(reason="y1 bf16"):
        for mt in range(2):
            nc.vector.tensor_scalar(y1b[:, mt, no:no + nn], nrm[:, mt, :nn],
                                    ln1g[:, mt:mt + 1], ln1b[:, mt:mt + 1],
                                    op0=ALU.mult, op1=ALU.add)


def _sweep_b_tile(nc, ppsA, p3t, y1b, hbF, w1T, b1, no, nn):
    for mt in range(4):
        ps = ppsA.tile([128, 512], F32, tag="psA")
        for kt in range(2):
            nc.tensor.matmul(ps[:, :nn], w1T[:, kt, mt * 128:(mt + 1) * 128],
                             y1b[:, kt, no:no + nn], start=(kt == 0),
                             stop=(kt == 1))
        nc.scalar.activation(hbF[:, mt, no:no + nn], ps[:, :nn], ACTF.Silu,
                             bias=b1[:, mt:mt + 1])


def _sweep_c_tile(nc, d, ppsA, ppsM, p3t, y1b, hbF, w2T, b2,
                  ln2g, ln2b, ones, negone, epsc, nhalf, no, nn):
    zt = p3t.tile([128, 2, 512], BF16, tag="zt3")
    for mt in range(2):
        ps = ppsA.tile([128, 512], F32, tag="psA")
        for kt in range(4):
            nc.tensor.matmul(ps[:, :nn], w2T[:, kt, mt * 128:(mt + 1) * 128],
                             hbF[:, kt, no:no + nn], start=(kt == 0),
                             stop=(kt == 3))
        with nc.allow_low_precision(reason="LN input bf16 ok"):
            # zt = (ps + b2) + y1b  (residual + bias in one op)
            nc.vector.scalar_tensor_tensor(zt[:, mt, :nn], ps[:, :nn],
                                           b2[:, mt:mt + 1],
                                           y1b[:, mt, no:no + nn],
                                           op0=ALU.add, op1=ALU.add)
    rep = _ln_norm(nc, ppsM, p3t, zt, ones, negone, epsc, nhalf, nn,
                   tail=True)
    nrm = _norm_apply(nc, p3t, zt, rep, nn)
    for mt in range(2):
        yo = p3t.tile([128, 512], F32, tag="yo")
        nc.vector.tensor_scalar(yo[:, :nn], nrm[:, mt, :nn],
                                ln2g[:, mt:mt + 1], ln2b[:, mt:mt + 1],
                                op0=ALU.mult, op1=ALU.add)
        nc.sync.dma_start(out=d["y_out"][mt, :, no:no + nn], in_=yo[:, :nn])


BF = ml_dtypes.bfloat16
F8 = ml_dtypes.float8_e4m3


def _j68_const():
    j = np.arange(WX, dtype=np.float32)
    grid = np.broadcast_to(j[:, None], (WX, P_PTS)).reshape(1, -1)
    return np.ascontiguousarray(np.broadcast_to(grid, (128, WX * P_PTS))
                                ).astype(np.float16)


def _maskW_const():
    n = np.arange(128)
    s = np.arange(SBLK)
    m = (n[:, None] % SBLK == s[None, :]).astype(np.float16)  # [n, s]
    return np.ascontiguousarray(
        np.repeat(m[:, :, None], WY * WX, axis=2).reshape(128, -1))


def _prep_inputs(inputs):
    f = np.asarray(inputs["feats"], np.float32)
    fp = np.asarray(inputs["feats_pos"], np.float32)
    anch = np.asarray(inputs["anchor_points"], np.float32)

    def bf(x):
        return np.asarray(x, np.float32).astype(BF)

    offW = np.asarray(inputs["off_W"], np.float32)
    attnW = np.asarray(inputs["attn_W"], np.float32)
    oab = np.concatenate([np.asarray(inputs["off_b"], np.float32),
                          np.asarray(inputs["attn_b"], np.float32)])
    # value bias folded through the out projection: agg gets +vb (the hat
    # weights per group sum to ~1), so out_b_eff = out_b + vb @ out_W
    outb_eff = (np.asarray(inputs["out_b"], np.float32)
                + np.asarray(inputs["value_b"], np.float32)
                @ np.asarray(inputs["out_W"], np.float32))
    shared = {
        "vW": bf(inputs["value_W"]),
        "oaW": bf(np.concatenate([offW, attnW], axis=1)),
        "oabrow": np.ascontiguousarray(oab.reshape(1, 192)).astype(BF),
        "outW": bf(inputs["out_W"]),
        "w1T": bf(np.asarray(inputs["ffn_w1"], np.float32).T),
        "b1": np.ascontiguousarray(
            np.asarray(inputs["ffn_b1"], np.float32).reshape(4, 128).T),
        "w2T": bf(np.asarray(inputs["ffn_w2"], np.float32).T),
        "b2": np.ascontiguousarray(
            np.asarray(inputs["ffn_b2"], np.float32).reshape(2, 128).T),
        "ln1g": np.ascontiguousarray(
            np.asarray(inputs["ln1_g"], np.float32).reshape(2, 128).T),
        "ln1b": np.ascontiguousarray(
            np.asarray(inputs["ln1_b"], np.float32).reshape(2, 128).T),
        "ln2g": np.ascontiguousarray(
            np.asarray(inputs["ln2_g"], np.float32).reshape(2, 128).T),
        "ln2b": np.ascontiguousarray(
            np.asarray(inputs["ln2_b"], np.float32).reshape(2, 128).T),
        "j68": _j68_const(),
        "identh": np.eye(128, dtype=np.float32).astype(np.float16),
        "maskW": _maskW_const(),
        "ones": np.full((128, 1), 1.0 / 256, np.float32).astype(BF),
        "ones128b": np.ones((1, 128), np.float32).astype(BF),
        "negone": np.full((1, 1), -1.0, np.float32).astype(BF),
        "epsc": np.full((1, 1), LN_EPS, np.float32),
        "nhalf": np.full((1, 1), -0.5, np.float32),
    }

    qsum = (f + fp).reshape(B, C, HW).astype(np.float32)
    qbias = qsum + outb_eff[None, :, None]
    in_maps = []
    for k in range(NCORES):
        b, s = k // 4, (k % 4) * NLOC
        q8 = qsum[b].astype(F8)
        ax = anch[b].reshape(HW, 2)[s:s + NLOC, 0] * W
        ay = anch[b].reshape(HW, 2)[s:s + NLOC, 1] * H
        ox = np.clip(np.rint(ax) - WX // 2, 0, W - WX)
        oy = np.clip(np.rint(ay) - WY // 2, 0, H - WY)
        axm = (ax + 0.5 - ox).astype(np.float32)
        aym = (ay + 0.5 - oy).astype(np.float32)
        m0 = (oy * W + ox).astype(np.int64)

        # gather idx i = blk*128 + p, p = s*NROW + dy*NDXP + dxp; idx points
        # at the 512B dx-pair (2 image positions) for position blk*16+s.
        idxs = np.zeros((NCH, NBLK, 128), np.int64)           # [c, blk, p]
        pos = m0.reshape(NCH, NBLK, SBLK)                     # [c, blk, s]
        for dy in range(WY):
            for dxp in range(NDXP):
                r = dy * NDXP + dxp
                idxs[:, :, np.arange(SBLK) * NROW + r] = pos + dy * W + 2 * dxp
        # wrapped-16: idx i lives at [i % 16, i // 16], replicated x8
        flat = idxs.reshape(NCH, NBLK * 128)                  # [c, i]
        g16 = flat.reshape(NCH, 64, 16).transpose(2, 0, 1)    # [lo, c, col]
        gidx = np.tile(g16.reshape(16, NCH * 64), (8, 1)).astype(np.int16)

        m = dict(shared)
        m["qsum8"] = np.ascontiguousarray(q8.reshape(2, 128, HW))
        m["qloc8"] = np.ascontiguousarray(
            q8[:, s:s + NLOC].reshape(2, 128, NLOC))
        m["qbin"] = np.ascontiguousarray(
            qbias[b, :, s:s + NLOC].reshape(2, 128, NLOC).astype(BF))
        axy2 = np.stack([axm.reshape(NCH, 128).T,
                         aym.reshape(NCH, 128).T], axis=2)
        m["axy2"] = np.ascontiguousarray(axy2.reshape(128, NCH * 2))
        m["gidx"] = gidx
        in_maps.append(m)
    return in_maps


def kernel(**inputs):
    if "nc" not in _CACHE:
        _CACHE["nc"] = _build_program()
    nc = _CACHE["nc"]
    in_maps = _prep_inputs(inputs)
    trace = bool(int(os.environ.get("KTRACE", "0")))
    res = run_bass_kernel_spmd(nc, in_maps, core_ids=list(range(NCORES)),
                               trace=trace)
    _CACHE["exec_time_ns"] = res.exec_time_ns
    _CACHE["trace"] = res.instructions_and_trace
    out = np.zeros((B, C, HW), np.float32)
    for k in range(NCORES):
        b, s = k // 4, (k % 4) * NLOC
        out[b, :, s:s + NLOC] = res.results[k]["y_out"].reshape(C, NLOC)
    return out.reshape(B, C, H, W)


# revision 86
# speedup vs baseline: 1.0014x; 1.0014x over previous
"""DeformTransformerBlock2D Trainium2 kernel (8-core SPMD, full I/O).

Sharding: core k handles batch k//4, image rows [20*(k%4), 20*(k%4)+20)
(3200 output positions). Each core computes the full-image value projection
for its batch (anchors are uniform-random, so the bilinear gather is global).

Bilinear gather: 4x4 round-centered window per position (clips only when
|offset| > 1 px; host-checked rel err 4.7e-3 vs the reference). One SWDGE
dma_gather per 128-position chunk fetches 8 dx-pair rows x 512B fp8 per
position: partition (s, dy, dxp) = s*8 + dy*2 + dxp, s in [0,16) within a
16-position block, 8 blocks per chunk -> exactly 128 partitions.

Weight pipeline (DVE fast modes): hm = min(|u-j|-1, 0) = -hat via
tensor_scalar (4x); tmp = hm_x*(hm_y*attn) via 2x tensor_tensor (signs
cancel); p-tree-reduce; per-(g,eo) PE transpose with an s-broadcast
stationary replicates the 8 window-rows into all 16 s-blocks; the
block-diag s-mask rides the single psum->SBUF staging multiply.

LN stats: mean/E[z^2] via PE ones-matmuls, variance folded in psum with a
-1 K=1 matmul of mean^2, rstd = exp(-0.5*ln(var+eps)) on Act (keeps the
whole kernel in the natural_log_exp act table except SiLU), stats
replicated across partitions by gpsimd.partition_broadcast so the
normalize runs at DVE 2x. Three act-table epochs total.
"""

import os
import numpy as np
import ml_dtypes

import concourse.bacc as bacc
import concourse.bass as bass
import concourse.tile as tile
from concourse import mybir
from concourse.bass_utils import run_bass_kernel_spmd

F32 = mybir.dt.float32
F16 = mybir.dt.float16
BF16 = mybir.dt.bfloat16
FP8 = mybir.dt.float8e4
I16 = mybir.dt.int16
AX = mybir.AxisListType
ALU = mybir.AluOpType
ACTF = mybir.ActivationFunctionType

B, C, H, W = 2, 256, 80, 160
G, P_PTS = 8, 8
HW = H * W                     # 12800
NCORES = 8
NLOC = 3200                    # positions per core
NCH = 25                       # chunks of 128 positions
WX, WY = 4, 4                  # gather window: 4 wide (2 dx-pairs), 4 tall
NDXP = WX // 2                 # dx pairs per row (512B gather elems)
NROW = WY * NDXP               # 8 (dy, dxp) rows per position
SBLK = 16                      # positions per matmul block
NBLK = 8                       # blocks per 128-chunk
LN_EPS = 1e-5

_CACHE = {}


def _nsplit(total, step):
    o, out = 0, []
    while o < total:
        out.append((o, min(step, total - o)))
        o += step
    return out


def _build_program():
    nc = bacc.Bacc("TRN2", target_bir_lowering=False, debug=False,
                   num_devices=NCORES)

    d = {}
    def din(name, shape, dt):
        d[name] = nc.dram_tensor(name, shape, dt, kind="ExternalInput")
    din("qsum8", (2, 128, HW), FP8)
    din("qloc8", (2, 128, NLOC), FP8)
    din("qbin", (2, 128, NLOC), BF16)     # residual, outb pre-folded (host)
    din("axy2", (128, NCH * 2), F32)
    din("j68", (128, WX * P_PTS), F16)    # j grid replicated over p
    din("gidx", (128, NCH * 64), I16)
    din("vW", (256, 256), BF16)
    din("oaW", (256, 192), BF16)
    din("oabrow", (1, 192), BF16)         # off/attn bias row (K=1 matmul)
    din("outW", (256, 256), BF16)
    din("w1T", (256, 512), BF16)
    din("b1", (128, 4), F32)
    din("w2T", (512, 256), BF16)
    din("b2", (128, 2), F32)
    din("ln1g", (128, 2), F32)
    din("ln1b", (128, 2), F32)
    din("ln2g", (128, 2), F32)
    din("ln2b", (128, 2), F32)
    din("identh", (128, 128), F16)
    din("maskW", (128, SBLK * WY * WX), F16)  # [n, s, (dy,dx)] = (n%16==s)
    din("ones", (128, 1), BF16)           # column of 1/256 (mean matmul)
    din("ones128b", (1, 128), BF16)       # row of ones (K=1 bias matmul)
    din("negone", (1, 1), BF16)           # -1 (variance fold)
    din("epsc", (1, 1), F32)              # LN epsilon (Act bias)
    din("nhalf", (1, 1), F32)             # -0.5 (Act scale for rstd)

    d["y_out"] = nc.dram_tensor("y_out", (2, 128, NLOC), F32,
                                kind="ExternalOutput")
    d["v8"] = nc.dram_tensor("v8scratch", (HW, 256), FP8)

    with tile.TileContext(nc) as tc:
        _emit(nc, tc, d)
    nc.compile()
    return nc


def _ld(nc, pool, dram, shape, dt, rearr=None, **rkw):
    t = pool.tile(shape, dt, tag="ld_" + dram.name)
    src = dram.ap()
    if rearr:
        src = src.rearrange(rearr, **rkw)
    nc.sync.dma_start(out=t, in_=src)
    return t


def _emit(nc, tc, d):
    import os as _os
    ABL = set(_os.environ.get("KABL", "").split(","))
    from contextlib import ExitStack
    ctx = ExitStack()
    pconst = ctx.enter_context(tc.tile_pool(name="pconst", bufs=1))
    pmain = ctx.enter_context(tc.tile_pool(name="pmain", bufs=1))
    ppsA = ctx.enter_context(tc.tile_pool(name="ppsA", bufs=2, space="PSUM"))
    ppsT = ctx.enter_context(tc.tile_pool(name="ppsT", bufs=1, space="PSUM"))

    # ---------- phase-1-critical input loads first ----------
    pin = ctx.enter_context(tc.tile_pool(name="pin", bufs=1))
    ql8 = _ld(nc, pin, d["qloc8"], [128, 2, NLOC], FP8, "kt k n -> k kt n")
    # ---------- constants ----------
    oaW = _ld(nc, pconst, d["oaW"], [128, 2, 192], BF16, "(kt k) m -> k kt m", k=128)
    oabrow = _ld(nc, pconst, d["oabrow"], [1, 192], BF16)
    ones128b = _ld(nc, pconst, d["ones128b"], [1, 128], BF16)
    vW = _ld(nc, pconst, d["vW"], [128, 2, 256], BF16, "(kt k) m -> k kt m", k=128)
    axy2 = _ld(nc, pconst, d["axy2"], [128, NCH, 2], F32)
    j68 = _ld(nc, pconst, d["j68"], [128, WX, P_PTS], F16)
    outW = _ld(nc, pconst, d["outW"], [128, 2, 256], BF16, "(kt k) m -> k kt m", k=128)
    w1T = _ld(nc, pconst, d["w1T"], [128, 2, 512], BF16, "(kt k) m -> k kt m", k=128)
    b1 = _ld(nc, pconst, d["b1"], [128, 4], F32)
    w2T = _ld(nc, pconst, d["w2T"], [128, 4, 256], BF16, "(kt k) m -> k kt m", k=128)
    b2 = _ld(nc, pconst, d["b2"], [128, 2], F32)
    ln1g = _ld(nc, pconst, d["ln1g"], [128, 2], F32)
    ln1b = _ld(nc, pconst, d["ln1b"], [128, 2], F32)
    ln2g = _ld(nc, pconst, d["ln2g"], [128, 2], F32)
    ln2b = _ld(nc, pconst, d["ln2b"], [128, 2], F32)
    gidx = _ld(nc, pconst, d["gidx"], [128, NCH * 64], I16)
    identh = _ld(nc, pconst, d["identh"], [128, 128], F16)
    maskW = _ld(nc, pconst, d["maskW"], [128, SBLK, WY * WX], F16)
    ones = _ld(nc, pconst, d["ones"], [128, 1], BF16)
    negone = _ld(nc, pconst, d["negone"], [1, 1], BF16)
    epsc = _ld(nc, pconst, d["epsc"], [1, 1], F32)
    nhalf = _ld(nc, pconst, d["nhalf"], [1, 1], F32)

    # ---------- persistent activations ----------
    qb = pmain.tile([128, 2, NLOC], BF16)
    aggT = pmain.tile([128, 2, NLOC], BF16)
    y1b = pmain.tile([128, 2, NLOC], BF16)
    hbF = pmain.tile([128, 4, NLOC], BF16)

    pmid_cm = tc.tile_pool(name="pmid", bufs=1)
    pmid = pmid_cm.__enter__()
    offa = pmid.tile([128, NCH, 192], F16)
    attnN = pmid.tile([128, NCH, G, P_PTS], F16)

    # ========== phase 1: value field + projections ==========
    with tc.tile_pool(name="ph1", bufs=1) as p1, \
         tc.tile_pool(name="ph1t", bufs=8) as p1t, \
         tc.tile_pool(name="ppsB", bufs=2, space="PSUM") as ppsB:
        nc.sync.dma_start(out=qb, in_=d["qbin"].ap()
                          .rearrange("kt k n -> k kt n"))
        qiap = d["qsum8"].ap().rearrange("kt k n -> k kt n")

        # off/attn projections, bias folded in via a K=1 matmul
        for c in range(NCH):
            ps = ppsB.tile([128, 192], F32, tag="psB")
            for kt in range(2):
                nc.tensor.matmul(ps, ql8[:, kt, c * 128:(c + 1) * 128],
                                 oaW[:, kt, :], start=(kt == 0), stop=False)
            nc.tensor.matmul(ps, ones128b, oabrow, start=False, stop=True)
            if c % 2 == 0:
                nc.vector.tensor_copy(out=offa[:, c], in_=ps)
            else:
                nc.scalar.activation(offa[:, c], ps, ACTF.Copy)
            # softmax over points
            ae = p1t.tile([128, G, P_PTS], F32, tag="ae")
            nc.scalar.activation(ae.rearrange("n g p -> n (g p)"),
                                 offa[:, c, 128:192], ACTF.Exp)
            ssum = p1t.tile([128, G], F32, tag="ssum")
            nc.vector.tensor_reduce(ssum, ae, axis=AX.X, op=ALU.add)
            srec = p1t.tile([128, G], F32, tag="srec")
            nc.vector.reciprocal(srec, ssum)
            nc.vector.tensor_mul(attnN[:, c], ae,
                                 srec.unsqueeze(2).broadcast_to([128, G, P_PTS]))

        # value projection, position-major output: lhsT = q chunk (fp8
        # stationary [128ch, 128pos]), rhs = vW row block -> psum[pos, 256ch]
        for pc in range(25 if "noph1v" not in ABL else 0):  # 512-px chunks
            no = pc * 512
            qc = p1t.tile([128, 2, 512], FP8, tag="qc")
            nc.sync.dma_start(out=qc, in_=qiap[:, :, no:no + 512])
            vrow = p1t.tile([128, 2, 2, 256], FP8, tag="vrow")
            for half in range(2):
                ps = ppsB.tile([128, 2, 256], F32, tag="psPM")
                for b in range(2):
                    blk = half * 2 + b
                    for kt in range(2):
                        nc.tensor.matmul(
                            ps[:, b], qc[:, kt, blk * 128:(blk + 1) * 128],
                            vW[:, kt, :], start=(kt == 0), stop=(kt == 1))
                nc.scalar.activation(
                    vrow[:, half].rearrange("n b c -> n (b c)"),
                    ps.rearrange("n b c -> n (b c)"), ACTF.Copy)
            v8out = bass.AP(tensor=d["v8"], offset=no * 256,
                            ap=[[256, 128], [128 * 256, 4], [1, 256]])
            nc.sync.dma_start(out=v8out, in_=vrow.rearrange(
                "n h b c -> n (h b) c"))

    # ========== phase 2 + sweep A (out-proj + LN1), interleaved ==========
    v8in = bass.AP(tensor=d["v8"], offset=0,
               ap=[[256, HW - 1], [1, 512]])
    with tc.tile_pool(name="ph2w", bufs=4) as p2w, \
         tc.tile_pool(name="ph2m", bufs=3) as p2m, \
         tc.tile_pool(name="ph2t", bufs=3) as p2t, \
         tc.tile_pool(name="ph3t", bufs=3) as p3t, \
         tc.tile_pool(name="ppsW", bufs=2, space="PSUM") as ppsW, \
         tc.tile_pool(name="ppsM", bufs=1, space="PSUM") as ppsM:
        done_tiles = []
        done_b = []
        done_c = []
        def flush_tiles(upto):
            for no, nn in _nsplit(NLOC, 512):
                if no + nn <= upto and (no, nn) not in done_tiles:
                    done_tiles.append((no, nn))
                    if "noph3" not in ABL:
                        _sweep_a_tile(nc, ppsA, ppsM, p3t, qb, aggT,
                                      y1b, outW, ln1g, ln1b, ones, negone,
                                      epsc, nhalf, no, nn)
            # once the first 4 tiles' LN1 is out, start their FFN1+SiLU
            # batch (one extra act-table round-trip, fills the B bubble)
            if len(done_tiles) >= 4 and not done_b and "noph3" not in ABL:
                for no, nn in _nsplit(NLOC, 512)[:4]:
                    done_b.append((no, nn))
                    _sweep_b_tile(nc, ppsA, p3t, y1b, hbF, w1T, b1, no, nn)
                for no, nn in _nsplit(NLOC, 512)[:3]:
                    done_c.append((no, nn))
                    _sweep_c_tile(nc, d, ppsA, ppsM, p3t, y1b, hbF, w2T,
                                  b2, ln2g, ln2b, ones, negone, epsc, nhalf,
                                  no, nn)
        wins = {}

        def issue_gather(c):
            # windows: partition (s, dy, dxp) holds, per 16-pos block, the
            # 512B dx-pair (2 taps x 256ch fp8) of position blk*16+s.
            win = p2w.tile([128, NBLK, 512], FP8, tag="win")
            nc.gpsimd.dma_gather(
                out_ap=win, in_ap=v8in,
                idxs_ap=gidx[:, c * 64:(c + 1) * 64],
                num_idxs=1024, num_idxs_reg=1024,
                elem_size=512, elem_step=256)
            wins[c] = win

        if "nogather" not in ABL:
            issue_gather(0)
            issue_gather(1)
            issue_gather(2)
        for c in range(NCH):
            flush_tiles(c * 128)
            if "nogather" in ABL:
                continue
            if c + 3 < NCH:
                issue_gather(c + 3)
            win = wins.pop(c)

            if "nowt" in ABL:
                continue
            # u = off + anchor (in-window coords), packed f16
            u = p2t.tile([128, 2, G, P_PTS], F16, tag="u")
            offc = offa[:, c, 0:128].rearrange(
                "n (g p two) -> n two g p", g=G, two=2)
            for xy in range(2):
                nc.vector.tensor_scalar(u[:, xy], offc[:, xy],
                                        axy2[:, c, xy:xy + 1], None,
                                        op0=ALU.add)
            # d = u - j over the 4-tap grid; hm = min(|d|-1, 0) = -hat
            dt_ = p2t.tile([128, 2 * G, WX, P_PTS], F16, tag="dt")
            nc.vector.tensor_tensor(
                dt_, u.rearrange("n two g p -> n (two g) p").unsqueeze(2)
                .broadcast_to([128, 2 * G, WX, P_PTS]),
                j68.unsqueeze(1).broadcast_to([128, 2 * G, WX, P_PTS]),
                op=ALU.subtract)
            dtf = dt_.rearrange("n ag j p -> n (ag j p)")
            t1 = p2t.tile([128, 2 * G * WX * P_PTS], F16, tag="t1")
            nc.vector.tensor_scalar(t1, dtf, -1.0, 2.0, op0=ALU.mult,
                                    op1=ALU.add)
            hm = p2t.tile([128, 2, G, WX, P_PTS], F16, tag="hm")
            hmf = hm.rearrange("n a g j p -> n (a g j p)")
            nc.vector.tensor_tensor(hmf, t1, dtf, op=ALU.min)
            nc.vector.tensor_scalar(hmf, hmf, 0.0, None, op0=ALU.max)
            # cy2 = hm_y * attn  (= -hat_y*attn, <= 0)
            cy2 = p2t.tile([128, G, WY, P_PTS], F16, tag="cy2")
            nc.vector.tensor_mul(
                cy2, hm[:, 1, :, 0:WY, :], attnN[:, c].unsqueeze(2)
                .broadcast_to([128, G, WY, P_PTS]))
            # tmp[g,dy,dx,p] = cy2[g,dy,p] * hm_x[g,dx,p]  (signs cancel)
            tmp = p2m.tile([128, G, WY, WX, P_PTS], F16, tag="fmatmp")
            nc.vector.tensor_mul(
                tmp, cy2.unsqueeze(3).broadcast_to([128, G, WY, WX, P_PTS]),
                hm[:, 0].unsqueeze(2).broadcast_to([128, G, WY, WX, P_PTS]))
            tf = tmp.rearrange("n g dy dx p -> n (g dy dx) p")
            nc.vector.tensor_add(tf[:, :, 0:4], tf[:, :, 0:4], tf[:, :, 4:8])
            nc.vector.tensor_add(tf[:, :, 0:2], tf[:, :, 0:2], tf[:, :, 2:4])
            cwb = p2m.tile([128, G, WY, WX], F16, tag="cwb")
            nc.vector.tensor_add(cwb.rearrange("n g dy dx -> n (g dy dx)"),
                                 tf[:, :, 0], tf[:, :, 1])
            # block-diag s-mask expansion in n-major (wide mask keeps 2x)
            cwm = p2m.tile([128, G, SBLK, WY * WX], F16, tag="cwm")
            cwbf = cwb.rearrange("n g dy dx -> n g (dy dx)")
            nc.vector.tensor_mul(
                cwm, cwbf.unsqueeze(2).broadcast_to([128, G, SBLK, WY * WX]),
                maskW.unsqueeze(1).broadcast_to([128, G, SBLK, WY * WX]))
            # transpose per (g, eo): pst[(s,dy,dxp), 2g+eo, n]
            pst = ppsW.tile([128, 2 * G, 128], F16, tag="cwT")
            cwmv = cwm.rearrange("n g s (dy dxp eo) -> n g eo s dy dxp",
                                 eo=2, dy=WY)
            for g in range(G):
                for eo in range(2):
                    nc.tensor.transpose(pst[:, 2 * g + eo, :],
                                        cwmv[:, g, eo], identh)
            # psum -> SBUF rhs staging, split across Act/DVE by parity
            cwsb = p2w.tile([128, 2 * G, 128], F16, tag="cwsb")
            if c % 3 == 2 or c >= 23:
                nc.vector.tensor_copy(out=cwsb, in_=pst)
            else:
                nc.scalar.copy(cwsb, pst)
            if "nofma" in ABL:
                continue
            # window contraction on the PE: windows stationary, weights moving
            psa = ppsT.tile([128, 2, 128], F32, tag="psagg")
            for blk in range(NBLK):
                for g in range(G):
                    kt, gq = g // 4, g % 4
                    for eo in range(2):
                        nc.tensor.matmul(
                            psa[gq * 32:(gq + 1) * 32, kt,
                                blk * SBLK:(blk + 1) * SBLK],
                            win[:, blk, eo * 256 + g * 32:
                                eo * 256 + (g + 1) * 32],
                            cwsb[:, 2 * g + eo, blk * SBLK:(blk + 1) * SBLK],
                            start=(eo == 0), stop=(eo == 1),
                            tile_position=(0, gq * 32))
            for kt in range(2):
                if c >= 23:
                    nc.vector.tensor_copy(
                        out=aggT[:, kt, c * 128:(c + 1) * 128],
                        in_=psa[:, kt, :])
                else:
                    nc.scalar.activation(aggT[:, kt, c * 128:(c + 1) * 128],
                                         psa[:, kt, :], ACTF.Copy)
        flush_tiles(NLOC)

        # ===== sweep B: FFN1 + SiLU (one act-table epoch) =====
        if "noph3" not in ABL:
            for no, nn in _nsplit(NLOC, 512):
                if (no, nn) not in done_b:
                    _sweep_b_tile(nc, ppsA, p3t, y1b, hbF, w1T, b1, no, nn)
            # ===== sweep C: FFN2 + LN2 + output =====
            for no, nn in _nsplit(NLOC, 512):
                if (no, nn) not in done_c:
                    _sweep_c_tile(nc, d, ppsA, ppsM, p3t, y1b, hbF, w2T,
                                  b2, ln2g, ln2b, ones, negone, epsc, nhalf,
                                  no, nn)

    pmid_cm.__exit__(None, None, None)

    if "noph3" in ABL:
        for kt in range(2):
            nc.sync.dma_start(out=d["y_out"][kt], in_=qb[:, kt])
    ctx.close()


def _ln_norm(nc, ppsM, p3t, zt, ones, negone, epsc, nhalf, nn,
             tail=False):
    """LN stats for zt [128, 2, nn] bf16 -> rep [128, 2, nn] f16 SBUF
    (partition-broadcast rows: [:,0]=rstd, [:,1]=mean)."""
    psmv = ppsM.tile([128, 512], F32, tag="psM")
    psm = psmv[0:1]
    psv = psmv[32:33]
    for kt in range(2):
        nc.tensor.matmul(psm[:1, :nn], ones, zt[:, kt, :nn],
                         start=(kt == 0), stop=(kt == 1),
                         tile_position=(0, 0))
    sqt = p3t.tile([128, 2, 512], BF16, tag="lnsq")
    eng = nc.vector if tail else nc.gpsimd
    eng.tensor_mul(sqt[:, :, :nn], zt[:, :, :nn], zt[:, :, :nn])
    m2b = p3t.tile([1, 512], BF16, tag="m2b")
    nc.scalar.activation(m2b[:1, :nn], psm[:1, :nn], ACTF.Square)
    for kt in range(2):
        nc.tensor.matmul(psv[:1, :nn], ones, sqt[:, kt, :nn],
                         start=(kt == 0), stop=False,
                         tile_position=(0, 32))
    nc.tensor.matmul(psv[:1, :nn], negone, m2b[:1, :nn],
                     start=False, stop=True, tile_position=(0, 32))
    sq = p3t.tile([1, 512], F32, tag="sq")
    nc.scalar.activation(sq[:1, :nn], psv[:1, :nn], ACTF.Sqrt, bias=epsc)
    stat = p3t.tile([1, 2, 512], F16, tag="stat")
    with nc.allow_low_precision(reason="LN stats fit f16"):
        nc.vector.reciprocal(stat[:1, 0, :nn], sq[:1, :nn])
        if tail:
            nc.vector.tensor_copy(out=stat[:1, 1, :nn], in_=psm[:1, :nn])
        else:
            nc.scalar.activation(stat[:1, 1, :nn], psm[:1, :nn], ACTF.Copy)
    rep = p3t.tile([128, 2, 512], F16, tag="rep")
    if nn == 512:
        nc.gpsimd.partition_broadcast(rep, stat[0:1])
    else:
        for j in range(2):
            nc.gpsimd.partition_broadcast(rep[:, j, :nn], stat[0:1, j, :nn])
    return rep


def _norm_apply(nc, p3t, zt, rep, nn):
    nrm = p3t.tile([128, 2, 512], F16, tag="nrm3")
    with nc.allow_low_precision(reason="LN normalize f16"):
        nc.vector.tensor_sub(
            nrm[:, :, :nn], zt[:, :, :nn],
            rep[:, 1:2, :nn].broadcast_to([128, 2, nn]))
        nc.vector.tensor_mul(
            nrm[:, :, :nn], nrm[:, :, :nn],
            rep[:, 0:1, :nn].broadcast_to([128, 2, nn]))
    return nrm


def _sweep_a_tile(nc, ppsA, ppsM, p3t, qb, aggT, y1b, outW,
                  ln1g, ln1b, ones, negone, epsc, nhalf, no, nn):
    """out-proj + residual + LN1 -> y1b (bf16) for positions [no, no+nn)."""
    zt = p3t.tile([128, 2, 512], BF16, tag="zt3")
    for mt in range(2):
        ps = ppsA.tile([128, 512], F32, tag="psA")
        for kt in range(2):
            nc.tensor.matmul(ps[:, :nn], outW[:, kt, mt * 128:(mt + 1) * 128],
                             aggT[:, kt, no:no + nn],
                             start=(kt == 0), stop=(kt == 1))
        with nc.allow_low_precision(reason="LN input bf16 ok"):
            nc.vector.tensor_add(zt[:, mt, :nn], ps[:, :nn],
                                 qb[:, mt, no:no + nn])
    rep = _ln_norm(nc, ppsM, p3t, zt, ones, negone, epsc, nhalf, nn)
    nrm = _norm_apply(nc, p3t, zt, rep, nn)
    with nc.allow_low_precision(reason="y1 bf16"):
        for mt in range(2):
            nc.vector.tensor_scalar(y1b[:, mt, no:no + nn], nrm[:, mt, :nn],
                                    ln1g[:, mt:mt + 1], ln1b[:, mt:mt + 1],
                                    op0=ALU.mult, op1=ALU.add)


def _sweep_b_tile(nc, ppsA, p3t, y1b, hbF, w1T, b1, no, nn):
    for mt in range(4):
        ps = ppsA.tile([128, 512], F32, tag="psA")
        for kt in range(2):
            nc.tensor.matmul(ps[:, :nn], w1T[:, kt, mt * 128:(mt + 1) * 128],
                             y1b[:, kt, no:no + nn], start=(kt == 0),
                             stop=(kt == 1))
        nc.scalar.activation(hbF[:, mt, no:no + nn], ps[:, :nn], ACTF.Silu,
                             bias=b1[:, mt:mt + 1])


def _sweep_c_tile(nc, d, ppsA, ppsM, p3t, y1b, hbF, w2T, b2,
                  ln2g, ln2b, ones, negone, epsc, nhalf, no, nn):
    zt = p3t.tile([128, 2, 512], BF16, tag="zt3")
    for mt in range(2):
        ps = ppsA.tile([128, 512], F32, tag="psA")
        for kt in range(4):
            nc.tensor.matmul(ps[:, :nn], w2T[:, kt, mt * 128:(mt + 1) * 128],
                             hbF[:, kt, no:no + nn], start=(kt == 0),
                             stop=(kt == 3))
        with nc.allow_low_precision(reason="LN input bf16 ok"):
            # zt = (ps + b2) + y1b  (residual + bias in one op)
            nc.vector.scalar_tensor_tensor(zt[:, mt, :nn], ps[:, :nn],
                                           b2[:, mt:mt + 1],
                                           y1b[:, mt, no:no + nn],
                                           op0=ALU.add, op1=ALU.add)
    rep = _ln_norm(nc, ppsM, p3t, zt, ones, negone, epsc, nhalf, nn,
                   tail=True)
    nrm = _norm_apply(nc, p3t, zt, rep, nn)
    for mt in range(2):
        yo = p3t.tile([128, 512], F32, tag="yo")
        nc.vector.tensor_scalar(yo[:, :nn], nrm[:, mt, :nn],
                                ln2g[:, mt:mt + 1], ln2b[:, mt:mt + 1],
                                op0=ALU.mult, op1=ALU.add)
        nc.sync.dma_start(out=d["y_out"][mt, :, no:no + nn], in_=yo[:, :nn])


BF = ml_dtypes.bfloat16
F8 = ml_dtypes.float8_e4m3


def _j68_const():
    j = np.arange(WX, dtype=np.float32)
    grid = np.broadcast_to(j[:, None], (WX, P_PTS)).reshape(1, -1)
    return np.ascontiguousarray(np.broadcast_to(grid, (128, WX * P_PTS))
                                ).astype(np.float16)


def _maskW_const():
    n = np.arange(128)
    s = np.arange(SBLK)
    m = (n[:, None] % SBLK == s[None, :]).astype(np.float16)  # [n, s]
    return np.ascontiguousarray(
        np.repeat(m[:, :, None], WY * WX, axis=2).reshape(128, -1))


def _prep_inputs(inputs):
    f = np.asarray(inputs["feats"], np.float32)
    fp = np.asarray(inputs["feats_pos"], np.float32)
    anch = np.asarray(inputs["anchor_points"], np.float32)

    def bf(x):
        return np.asarray(x, np.float32).astype(BF)

    offW = np.asarray(inputs["off_W"], np.float32)
    attnW = np.asarray(inputs["attn_W"], np.float32)
    oab = np.concatenate([np.asarray(inputs["off_b"], np.float32),
                          np.asarray(inputs["attn_b"], np.float32)])
    # value bias folded through the out projection: agg gets +vb (the hat
    # weights per group sum to ~1), so out_b_eff = out_b + vb @ out_W
    outb_eff = (np.asarray(inputs["out_b"], np.float32)
                + np.asarray(inputs["value_b"], np.float32)
                @ np.asarray(inputs["out_W"], np.float32))
    shared = {
        "vW": bf(inputs["value_W"]),
        "oaW": bf(np.concatenate([offW, attnW], axis=1)),
        "oabrow": np.ascontiguousarray(oab.reshape(1, 192)).astype(BF),
        "outW": bf(inputs["out_W"]),
        "w1T": bf(np.asarray(inputs["ffn_w1"], np.float32).T),
        "b1": np.ascontiguousarray(
            np.asarray(inputs["ffn_b1"], np.float32).reshape(4, 128).T),
        "w2T": bf(np.asarray(inputs["ffn_w2"], np.float32).T),
        "b2": np.ascontiguousarray(
            np.asarray(inputs["ffn_b2"], np.float32).reshape(2, 128).T),
        "ln1g": np.ascontiguousarray(
            np.asarray(inputs["ln1_g"], np.float32).reshape(2, 128).T),
        "ln1b": np.ascontiguousarray(
            np.asarray(inputs["ln1_b"], np.float32).reshape(2, 128).T),
        "ln2g": np.ascontiguousarray(
            np.asarray(inputs["ln2_g"], np.float32).reshape(2, 128).T),
        "ln2b": np.ascontiguousarray(
            np.asarray(inputs["ln2_b"], np.float32).reshape(2, 128).T),
        "j68": _j68_const(),
        "identh": np.eye(128, dtype=np.float32).astype(np.float16),
        "maskW": _maskW_const(),
        "ones": np.full((128, 1), 1.0 / 256, np.float32).astype(BF),
        "ones128b": np.ones((1, 128), np.float32).astype(BF),
        "negone": np.full((1, 1), -1.0, np.float32).astype(BF),
        "epsc": np.full((1, 1), LN_EPS, np.float32),
        "nhalf": np.full((1, 1), -0.5, np.float32),
    }

    qsum = (f + fp).reshape(B, C, HW).astype(np.float32)
    qbias = qsum + outb_eff[None, :, None]
    in_maps = []
    for k in range(NCORES):
        b, s = k // 4, (k % 4) * NLOC
        q8 = qsum[b].astype(F8)
        ax = anch[b].reshape(HW, 2)[s:s + NLOC, 0] * W
        ay = anch[b].reshape(HW, 2)[s:s + NLOC, 1] * H
        ox = np.clip(np.rint(ax) - WX // 2, 0, W - WX)
        oy = np.clip(np.rint(ay) - WY // 2, 0, H - WY)
        axm = (ax + 0.5 - ox).astype(np.float32)
        aym = (ay + 0.5 - oy).astype(np.float32)
        m0 = (oy * W + ox).astype(np.int64)

        # gather idx i = blk*128 + p, p = s*NROW + dy*NDXP + dxp; idx points
        # at the 512B dx-pair (2 image positions) for position blk*16+s.
        idxs = np.zeros((NCH, NBLK, 128), np.int64)           # [c, blk, p]
        pos = m0.reshape(NCH, NBLK, SBLK)                     # [c, blk, s]
        for dy in range(WY):
            for dxp in range(NDXP):
                r = dy * NDXP + dxp
                idxs[:, :, np.arange(SBLK) * NROW + r] = pos + dy * W + 2 * dxp
        # wrapped-16: idx i lives at [i % 16, i // 16], replicated x8
        flat = idxs.reshape(NCH, NBLK * 128)                  # [c, i]
        g16 = flat.reshape(NCH, 64, 16).transpose(2, 0, 1)    # [lo, c, col]
        gidx = np.tile(g16.reshape(16, NCH * 64), (8, 1)).astype(np.int16)

        m = dict(shared)
        m["qsum8"] = np.ascontiguousarray(q8.reshape(2, 128, HW))
        m["qloc8"] = np.ascontiguousarray(
            q8[:, s:s + NLOC].reshape(2, 128, NLOC))
        m["qbin"] = np.ascontiguousarray(
            qbias[b, :, s:s + NLOC].reshape(2, 128, NLOC).astype(BF))
        axy2 = np.stack([axm.reshape(NCH, 128).T,
                         aym.reshape(NCH, 128).T], axis=2)
        m["axy2"] = np.ascontiguousarray(axy2.reshape(128, NCH * 2))
        m["gidx"] = gidx
        in_maps.append(m)
    return in_maps


def kernel(**inputs):
    if "nc" not in _CACHE:
        _CACHE["nc"] = _build_program()
    nc = _CACHE["nc"]
    in_maps = _prep_inputs(inputs)
    trace = bool(int(os.environ.get("KTRACE", "0")))
    res = run_bass_kernel_spmd(nc, in_maps, core_ids=list(range(NCORES)),
                               trace=trace)
    _CACHE["exec_time_ns"] = res.exec_time_ns
    _CACHE["trace"] = res.instructions_and_trace
    out = np.zeros((B, C, HW), np.float32)
    for k in range(NCORES):
        b, s = k // 4, (k % 4) * NLOC
        out[b, :, s:s + NLOC] = res.results[k]["y_out"].reshape(C, NLOC)
    return out.reshape(B, C, H, W)


# revision 89
# speedup vs baseline: 1.0029x; 1.0014x over previous
"""DeformTransformerBlock2D Trainium2 kernel (8-core SPMD, full I/O).

Sharding: core k handles batch k//4, image rows [20*(k%4), 20*(k%4)+20)
(3200 output positions). Each core computes the full-image value projection
for its batch (anchors are uniform-random, so the bilinear gather is global).

Bilinear gather: 4x4 round-centered window per position (clips only when
|offset| > 1 px; host-checked rel err 4.7e-3 vs the reference). One SWDGE
dma_gather per 128-position chunk fetches 8 dx-pair rows x 512B fp8 per
position: partition (s, dy, dxp) = s*8 + dy*2 + dxp, s in [0,16) within a
16-position block, 8 blocks per chunk -> exactly 128 partitions.

Weight pipeline (DVE fast modes): hm = min(|u-j|-1, 0) = -hat via
tensor_scalar (4x); tmp = hm_x*(hm_y*attn) via 2x tensor_tensor (signs
cancel); p-tree-reduce; per-(g,eo) PE transpose with an s-broadcast
stationary replicates the 8 window-rows into all 16 s-blocks; the
block-diag s-mask rides the single psum->SBUF staging multiply.

LN stats: mean/E[z^2] via PE ones-matmuls, variance folded in psum with a
-1 K=1 matmul of mean^2, rstd = exp(-0.5*ln(var+eps)) on Act (keeps the
whole kernel in the natural_log_exp act table except SiLU), stats
replicated across partitions by gpsimd.partition_broadcast so the
normalize runs at DVE 2x. Three act-table epochs total.
"""

import os
import numpy as np
import ml_dtypes

import concourse.bacc as bacc
import concourse.bass as bass
import concourse.tile as tile
from concourse import mybir
from concourse.bass_utils import run_bass_kernel_spmd

F32 = mybir.dt.float32
F16 = mybir.dt.float16
BF16 = mybir.dt.bfloat16
FP8 = mybir.dt.float8e4
I16 = mybir.dt.int16
AX = mybir.AxisListType
ALU = mybir.AluOpType
ACTF = mybir.ActivationFunctionType

B, C, H, W = 2, 256, 80, 160
G, P_PTS = 8, 8
HW = H * W                     # 12800
NCORES = 8
NLOC = 3200                    # positions per core
NCH = 25                       # chunks of 128 positions
WX, WY = 4, 4                  # gather window: 4 wide (2 dx-pairs), 4 tall
NDXP = WX // 2                 # dx pairs per row (512B gather elems)
NROW = WY * NDXP               # 8 (dy, dxp) rows per position
SBLK = 16                      # positions per matmul block
NBLK = 8                       # blocks per 128-chunk
LN_EPS = 1e-5

_CACHE = {}


def _nsplit(total, step):
    o, out = 0, []
    while o < total:
        out.append((o, min(step, total - o)))
        o += step
    return out


def _build_program():
    nc = bacc.Bacc("TRN2", target_bir_lowering=False, debug=False,
                   num_devices=NCORES)

    d = {}
    def din(name, shape, dt):
        d[name] = nc.dram_tensor(name, shape, dt, kind="ExternalInput")
    din("qsum8", (2, 128, HW), FP8)
    din("qloc8", (2, 128, NLOC), FP8)
    din("qbin", (2, 128, NLOC), BF16)     # residual, outb pre-folded (host)
    din("axy2", (128, NCH * 2), F32)
    din("j68", (128, WX * P_PTS), F16)    # j grid replicated over p
    din("gidx", (128, NCH * 64), I16)
    din("vW", (256, 256), BF16)
    din("oaW", (256, 192), BF16)
    din("oabrow", (1, 192), BF16)         # off/attn bias row (K=1 matmul)
    din("outW", (256, 256), BF16)
    din("w1T", (256, 512), BF16)
    din("b1", (128, 4), F32)
    din("w2T", (512, 256), BF16)
    din("b2", (128, 2), F32)
    din("ln1g", (128, 2), F32)
    din("ln1b", (128, 2), F32)
    din("ln2g", (128, 2), F32)
    din("ln2b", (128, 2), F32)
    din("identh", (128, 128), F16)
    din("maskW", (128, SBLK * WY * WX), F16)  # [n, s, (dy,dx)] = (n%16==s)
    din("ones", (128, 1), BF16)           # column of 1/256 (mean matmul)
    din("ones128b", (1, 128), BF16)       # row of ones (K=1 bias matmul)
    din("negone", (1, 1), BF16)           # -1 (variance fold)
    din("epsc", (1, 1), F32)              # LN epsilon (Act bias)
    din("nhalf", (1, 1), F32)             # -0.5 (Act scale for rstd)

    d["y_out"] = nc.dram_tensor("y_out", (2, 128, NLOC), F32,
                                kind="ExternalOutput")
    d["v8"] = nc.dram_tensor("v8scratch", (HW, 256), FP8)

    with tile.TileContext(nc) as tc:
        _emit(nc, tc, d)
    nc.compile()
    return nc


def _ld(nc, pool, dram, shape, dt, rearr=None, **rkw):
    t = pool.tile(shape, dt, tag="ld_" + dram.name)
    src = dram.ap()
    if rearr:
        src = src.rearrange(rearr, **rkw)
    nc.sync.dma_start(out=t, in_=src)
    return t


def _emit(nc, tc, d):
    import os as _os
    ABL = set(_os.environ.get("KABL", "").split(","))
    from contextlib import ExitStack
    ctx = ExitStack()
    pconst = ctx.enter_context(tc.tile_pool(name="pconst", bufs=1))
    pmain = ctx.enter_context(tc.tile_pool(name="pmain", bufs=1))
    ppsA = ctx.enter_context(tc.tile_pool(name="ppsA", bufs=2, space="PSUM"))
    ppsT = ctx.enter_context(tc.tile_pool(name="ppsT", bufs=1, space="PSUM"))

    # ---------- phase-1-critical input loads first ----------
    pin = ctx.enter_context(tc.tile_pool(name="pin", bufs=1))
    ql8 = _ld(nc, pin, d["qloc8"], [128, 2, NLOC], FP8, "kt k n -> k kt n")
    # ---------- constants ----------
    oaW = _ld(nc, pconst, d["oaW"], [128, 2, 192], BF16, "(kt k) m -> k kt m", k=128)
    oabrow = _ld(nc, pconst, d["oabrow"], [1, 192], BF16)
    ones128b = _ld(nc, pconst, d["ones128b"], [1, 128], BF16)
    vW = _ld(nc, pconst, d["vW"], [128, 2, 256], BF16, "(kt k) m -> k kt m", k=128)
    axy2 = _ld(nc, pconst, d["axy2"], [128, NCH, 2], F32)
    j68 = _ld(nc, pconst, d["j68"], [128, WX, P_PTS], F16)
    outW = _ld(nc, pconst, d["outW"], [128, 2, 256], BF16, "(kt k) m -> k kt m", k=128)
    w1T = _ld(nc, pconst, d["w1T"], [128, 2, 512], BF16, "(kt k) m -> k kt m", k=128)
    b1 = _ld(nc, pconst, d["b1"], [128, 4], F32)
    w2T = _ld(nc, pconst, d["w2T"], [128, 4, 256], BF16, "(kt k) m -> k kt m", k=128)
    b2 = _ld(nc, pconst, d["b2"], [128, 2], F32)
    ln1g = _ld(nc, pconst, d["ln1g"], [128, 2], F32)
    ln1b = _ld(nc, pconst, d["ln1b"], [128, 2], F32)
    ln2g = _ld(nc, pconst, d["ln2g"], [128, 2], F32)
    ln2b = _ld(nc, pconst, d["ln2b"], [128, 2], F32)
    gidx = _ld(nc, pconst, d["gidx"], [128, NCH * 64], I16)
    identh = _ld(nc, pconst, d["identh"], [128, 128], F16)
    maskW = _ld(nc, pconst, d["maskW"], [128, SBLK, WY * WX], F16)
    ones = _ld(nc, pconst, d["ones"], [128, 1], BF16)
    negone = _ld(nc, pconst, d["negone"], [1, 1], BF16)
    epsc = _ld(nc, pconst, d["epsc"], [1, 1], F32)
    nhalf = _ld(nc, pconst, d["nhalf"], [1, 1], F32)

    # ---------- persistent activations ----------
    qb = pmain.tile([128, 2, NLOC], BF16)
    aggT = pmain.tile([128, 2, NLOC], BF16)
    y1b = pmain.tile([128, 2, NLOC], BF16)
    hbF = pmain.tile([128, 4, NLOC], BF16)

    pmid_cm = tc.tile_pool(name="pmid", bufs=1)
    pmid = pmid_cm.__enter__()
    offa = pmid.tile([128, NCH, 192], F16)
    attnN = pmid.tile([128, NCH, G, P_PTS], F16)

    # ========== phase 1: value field + projections ==========
    with tc.tile_pool(name="ph1", bufs=1) as p1, \
         tc.tile_pool(name="ph1t", bufs=12) as p1t, \
         tc.tile_pool(name="ppsB", bufs=2, space="PSUM") as ppsB:
        nc.sync.dma_start(out=qb, in_=d["qbin"].ap()
                          .rearrange("kt k n -> k kt n"))
        qiap = d["qsum8"].ap().rearrange("kt k n -> k kt n")

        # off/attn projections, bias folded in via a K=1 matmul
        for c in range(NCH):
            ps = ppsB.tile([128, 192], F32, tag="psB")
            for kt in range(2):
                nc.tensor.matmul(ps, ql8[:, kt, c * 128:(c + 1) * 128],
                                 oaW[:, kt, :], start=(kt == 0), stop=False)
            nc.tensor.matmul(ps, ones128b, oabrow, start=False, stop=True)
            if c % 2 == 0:
                nc.vector.tensor_copy(out=offa[:, c], in_=ps)
            else:
                nc.scalar.activation(offa[:, c], ps, ACTF.Copy)
            # softmax over points
            ae = p1t.tile([128, G, P_PTS], F32, tag="ae")
            nc.scalar.activation(ae.rearrange("n g p -> n (g p)"),
                                 offa[:, c, 128:192], ACTF.Exp)
            ssum = p1t.tile([128, G], F32, tag="ssum")
            nc.vector.tensor_reduce(ssum, ae, axis=AX.X, op=ALU.add)
            srec = p1t.tile([128, G], F32, tag="srec")
            nc.vector.reciprocal(srec, ssum)
            nc.vector.tensor_mul(attnN[:, c], ae,
                                 srec.unsqueeze(2).broadcast_to([128, G, P_PTS]))

        # value projection, position-major output: lhsT = q chunk (fp8
        # stationary [128ch, 128pos]), rhs = vW row block -> psum[pos, 256ch]
        for pc in range(25 if "noph1v" not in ABL else 0):  # 512-px chunks
            no = pc * 512
            qc = p1t.tile([128, 2, 512], FP8, tag="qc")
            nc.sync.dma_start(out=qc, in_=qiap[:, :, no:no + 512])
            vrow = p1t.tile([128, 2, 2, 256], FP8, tag="vrow")
            for half in range(2):
                ps = ppsB.tile([128, 2, 256], F32, tag="psPM")
                for b in range(2):
                    blk = half * 2 + b
                    for kt in range(2):
                        nc.tensor.matmul(
                            ps[:, b], qc[:, kt, blk * 128:(blk + 1) * 128],
                            vW[:, kt, :], start=(kt == 0), stop=(kt == 1))
                nc.scalar.activation(
                    vrow[:, half].rearrange("n b c -> n (b c)"),
                    ps.rearrange("n b c -> n (b c)"), ACTF.Copy)
            v8out = bass.AP(tensor=d["v8"], offset=no * 256,
                            ap=[[256, 128], [128 * 256, 4], [1, 256]])
            nc.sync.dma_start(out=v8out, in_=vrow.rearrange(
                "n h b c -> n (h b) c"))

    # ========== phase 2 + sweep A (out-proj + LN1), interleaved ==========
    v8in = bass.AP(tensor=d["v8"], offset=0,
               ap=[[256, HW - 1], [1, 512]])
    with tc.tile_pool(name="ph2w", bufs=4) as p2w, \
         tc.tile_pool(name="ph2m", bufs=3) as p2m, \
         tc.tile_pool(name="ph2t", bufs=3) as p2t, \
         tc.tile_pool(name="ph3t", bufs=3) as p3t, \
         tc.tile_pool(name="ppsW", bufs=2, space="PSUM") as ppsW, \
         tc.tile_pool(name="ppsM", bufs=1, space="PSUM") as ppsM:
        done_tiles = []
        done_b = []
        done_c = []
        def flush_tiles(upto):
            for no, nn in _nsplit(NLOC, 512):
                if no + nn <= upto and (no, nn) not in done_tiles:
                    done_tiles.append((no, nn))
                    if "noph3" not in ABL:
                        _sweep_a_tile(nc, ppsA, ppsM, p3t, qb, aggT,
                                      y1b, outW, ln1g, ln1b, ones, negone,
                                      epsc, nhalf, no, nn)
            # once the first 4 tiles' LN1 is out, start their FFN1+SiLU
            # batch (one extra act-table round-trip, fills the B bubble)
            if len(done_tiles) >= 4 and not done_b and "noph3" not in ABL:
                for no, nn in _nsplit(NLOC, 512)[:4]:
                    done_b.append((no, nn))
                    _sweep_b_tile(nc, ppsA, p3t, y1b, hbF, w1T, b1, no, nn)
                for no, nn in _nsplit(NLOC, 512)[:3]:
                    done_c.append((no, nn))
                    _sweep_c_tile(nc, d, ppsA, ppsM, p3t, y1b, hbF, w2T,
                                  b2, ln2g, ln2b, ones, negone, epsc, nhalf,
                                  no, nn)
        wins = {}

        def issue_gather(c):
            # windows: partition (s, dy, dxp) holds, per 16-pos block, the
            # 512B dx-pair (2 taps x 256ch fp8) of position blk*16+s.
            win = p2w.tile([128, NBLK, 512], FP8, tag="win")
            nc.gpsimd.dma_gather(
                out_ap=win, in_ap=v8in,
                idxs_ap=gidx[:, c * 64:(c + 1) * 64],
                num_idxs=1024, num_idxs_reg=1024,
                elem_size=512, elem_step=256)
            wins[c] = win

        if "nogather" not in ABL:
            issue_gather(0)
            issue_gather(1)
            issue_gather(2)
        for c in range(NCH):
            flush_tiles(c * 128)
            if "nogather" in ABL:
                continue
            if c + 3 < NCH:
                issue_gather(c + 3)
            win = wins.pop(c)

            if "nowt" in ABL:
                continue
            # u = off + anchor (in-window coords), packed f16
            u = p2t.tile([128, 2, G, P_PTS], F16, tag="u")
            offc = offa[:, c, 0:128].rearrange(
                "n (g p two) -> n two g p", g=G, two=2)
            for xy in range(2):
                nc.vector.tensor_scalar(u[:, xy], offc[:, xy],
                                        axy2[:, c, xy:xy + 1], None,
                                        op0=ALU.add)
            # d = u - j over the 4-tap grid; hm = min(|d|-1, 0) = -hat
            dt_ = p2t.tile([128, 2 * G, WX, P_PTS], F16, tag="dt")
            nc.vector.tensor_tensor(
                dt_, u.rearrange("n two g p -> n (two g) p").unsqueeze(2)
                .broadcast_to([128, 2 * G, WX, P_PTS]),
                j68.unsqueeze(1).broadcast_to([128, 2 * G, WX, P_PTS]),
                op=ALU.subtract)
            dtf = dt_.rearrange("n ag j p -> n (ag j p)")
            t1 = p2t.tile([128, 2 * G * WX * P_PTS], F16, tag="t1")
            nc.vector.tensor_scalar(t1, dtf, -1.0, 2.0, op0=ALU.mult,
                                    op1=ALU.add)
            hm = p2t.tile([128, 2, G, WX, P_PTS], F16, tag="hm")
            hmf = hm.rearrange("n a g j p -> n (a g j p)")
            nc.vector.tensor_tensor(hmf, t1, dtf, op=ALU.min)
            nc.vector.tensor_scalar(hmf, hmf, 0.0, None, op0=ALU.max)
            # cy2 = hm_y * attn  (= -hat_y*attn, <= 0)
            cy2 = p2t.tile([128, G, WY, P_PTS], F16, tag="cy2")
            nc.vector.tensor_mul(
                cy2, hm[:, 1, :, 0:WY, :], attnN[:, c].unsqueeze(2)
                .broadcast_to([128, G, WY, P_PTS]))
            # tmp[g,dy,dx,p] = cy2[g,dy,p] * hm_x[g,dx,p]  (signs cancel)
            tmp = p2m.tile([128, G, WY, WX, P_PTS], F16, tag="fmatmp")
            nc.vector.tensor_mul(
                tmp, cy2.unsqueeze(3).broadcast_to([128, G, WY, WX, P_PTS]),
                hm[:, 0].unsqueeze(2).broadcast_to([128, G, WY, WX, P_PTS]))
            tf = tmp.rearrange("n g dy dx p -> n (g dy dx) p")
            nc.vector.tensor_add(tf[:, :, 0:4], tf[:, :, 0:4], tf[:, :, 4:8])
            nc.vector.tensor_add(tf[:, :, 0:2], tf[:, :, 0:2], tf[:, :, 2:4])
            cwb = p2m.tile([128, G, WY, WX], F16, tag="cwb")
            nc.vector.tensor_add(cwb.rearrange("n g dy dx -> n (g dy dx)"),
                                 tf[:, :, 0], tf[:, :, 1])
            # block-diag s-mask expansion in n-major (wide mask keeps 2x)
            cwm = p2m.tile([128, G, SBLK, WY * WX], F16, tag="cwm")
            cwbf = cwb.rearrange("n g dy dx -> n g (dy dx)")
            nc.vector.tensor_mul(
                cwm, cwbf.unsqueeze(2).broadcast_to([128, G, SBLK, WY * WX]),
                maskW.unsqueeze(1).broadcast_to([128, G, SBLK, WY * WX]))
            # transpose per (g, eo): pst[(s,dy,dxp), 2g+eo, n]
            pst = ppsW.tile([128, 2 * G, 128], F16, tag="cwT")
            cwmv = cwm.rearrange("n g s (dy dxp eo) -> n g eo s dy dxp",
                                 eo=2, dy=WY)
            for g in range(G):
                for eo in range(2):
                    nc.tensor.transpose(pst[:, 2 * g + eo, :],
                                        cwmv[:, g, eo], identh)
            # psum -> SBUF rhs staging, split across Act/DVE by parity
            cwsb = p2w.tile([128, 2 * G, 128], F16, tag="cwsb")
            if c % 3 == 2 or c >= 23:
                nc.vector.tensor_copy(out=cwsb, in_=pst)
            else:
                nc.scalar.copy(cwsb, pst)
            if "nofma" in ABL:
                continue
            # window contraction on the PE: windows stationary, weights moving
            psa = ppsT.tile([128, 2, 128], F32, tag="psagg")
            for blk in range(NBLK):
                for g in range(G):
                    kt, gq = g // 4, g % 4
                    for eo in range(2):
                        nc.tensor.matmul(
                            psa[gq * 32:(gq + 1) * 32, kt,
                                blk * SBLK:(blk + 1) * SBLK],
                            win[:, blk, eo * 256 + g * 32:
                                eo * 256 + (g + 1) * 32],
                            cwsb[:, 2 * g + eo, blk * SBLK:(blk + 1) * SBLK],
                            start=(eo == 0), stop=(eo == 1),
                            tile_position=(0, gq * 32))
            for kt in range(2):
                if c >= 23:
                    nc.vector.tensor_copy(
                        out=aggT[:, kt, c * 128:(c + 1) * 128],
                        in_=psa[:, kt, :])
                else:
                    nc.scalar.activation(aggT[:, kt, c * 128:(c + 1) * 128],
                                         psa[:, kt, :], ACTF.Copy)
        flush_tiles(NLOC)

        # ===== sweep B: FFN1 + SiLU (one act-table epoch) =====
        if "noph3" not in ABL:
            for no, nn in _nsplit(NLOC, 512):
                if (no, nn) not in done_b:
                    _sweep_b_tile(nc, ppsA, p3t, y1b, hbF, w1T, b1, no, nn)
            # ===== sweep C: FFN2 + LN2 + output =====
            for no, nn in _nsplit(NLOC, 512):
                if (no, nn) not in done_c:
                    _sweep_c_tile(nc, d, ppsA, ppsM, p3t, y1b, hbF, w2T,
                                  b2, ln2g, ln2b, ones, negone, epsc, nhalf,
                                  no, nn)

    pmid_cm.__exit__(None, None, None)

    if "noph3" in ABL:
        for kt in range(2):
            nc.sync.dma_start(out=d["y_out"][kt], in_=qb[:, kt])
    ctx.close()


def _ln_norm(nc, ppsM, p3t, zt, ones, negone, epsc, nhalf, nn,
             tail=False):
    """LN stats for zt [128, 2, nn] bf16 -> rep [128, 2, nn] f16 SBUF
    (partition-broadcast rows: [:,0]=rstd, [:,1]=mean)."""
    psmv = ppsM.tile([128, 512], F32, tag="psM")
    psm = psmv[0:1]
    psv = psmv[32:33]
    for kt in range(2):
        nc.tensor.matmul(psm[:1, :nn], ones, zt[:, kt, :nn],
                         start=(kt == 0), stop=(kt == 1),
                         tile_position=(0, 0))
    sqt = p3t.tile([128, 2, 512], BF16, tag="lnsq")
    eng = nc.vector if tail else nc.gpsimd
    eng.tensor_mul(sqt[:, :, :nn], zt[:, :, :nn], zt[:, :, :nn])
    m2b = p3t.tile([1, 512], BF16, tag="m2b")
    nc.scalar.activation(m2b[:1, :nn], psm[:1, :nn], ACTF.Square)
    for kt in range(2):
        nc.tensor.matmul(psv[:1, :nn], ones, sqt[:, kt, :nn],
                         start=(kt == 0), stop=False,
                         tile_position=(0, 32))
    nc.tensor.matmul(psv[:1, :nn], negone, m2b[:1, :nn],
                     start=False, stop=True, tile_position=(0, 32))
    sq = p3t.tile([1, 512], F32, tag="sq")
    nc.scalar.activation(sq[:1, :nn], psv[:1, :nn], ACTF.Sqrt, bias=epsc)
    stat = p3t.tile([1, 2, 512], F16, tag="stat")
    with nc.allow_low_precision(reason="LN stats fit f16"):
        nc.vector.reciprocal(stat[:1, 0, :nn], sq[:1, :nn])
        if tail:
            nc.vector.tensor_copy(out=stat[:1, 1, :nn], in_=psm[:1, :nn])
        else:
            nc.scalar.activation(stat[:1, 1, :nn], psm[:1, :nn], ACTF.Copy)
    rep = p3t.tile([128, 2, 512], F16, tag="rep")
    if nn == 512:
        nc.gpsimd.partition_broadcast(rep, stat[0:1])
    else:
        for j in range(2):
            nc.gpsimd.partition_broadcast(rep[:, j, :nn], stat[0:1, j, :nn])
    return rep


def _norm_apply(nc, p3t, zt, rep, nn):
    nrm = p3t.tile([128, 2, 512], F16, tag="nrm3")
    with nc.allow_low_precision(reason="LN normalize f16"):
        nc.vector.tensor_sub(
            nrm[:, :, :nn], zt[:, :, :nn],
            rep[:, 1:2, :nn].broadcast_to([128, 2, nn]))
        nc.vector.tensor_mul(
            nrm[:, :, :nn], nrm[:, :, :nn],
            rep[:, 0:1, :nn].broadcast_to([128, 2, nn]))
    return nrm


def _sweep_a_tile(nc, ppsA, ppsM, p3t, qb, aggT, y1b, outW,
                  ln1g, ln1b, ones, negone, epsc, nhalf, no, nn):
    """out-proj + residual + LN1 -> y1b (bf16) for positions [no, no+nn)."""
    zt = p3t.tile([128, 2, 512], BF16, tag="zt3")
    for mt in range(2):
        ps = ppsA.tile([128, 512], F32, tag="psA")
        for kt in range(2):
            nc.tensor.matmul(ps[:, :nn], outW[:, kt, mt * 128:(mt + 1) * 128],
                             aggT[:, kt, no:no + nn],
                             start=(kt == 0), stop=(kt == 1))
        with nc.allow_low_precision(reason="LN input bf16 ok"):
            nc.vector.tensor_add(zt[:, mt, :nn], ps[:, :nn],
                                 qb[:, mt, no:no + nn])
    rep = _ln_norm(nc, ppsM, p3t, zt, ones, negone, epsc, nhalf, nn)
    nrm = _norm_apply(nc, p3t, zt, rep, nn)
    with nc.allow_low_precision(reason="y1 bf16"):
        for mt in range(2):
            nc.vector.tensor_scalar(y1b[:, mt, no:no + nn], nrm[:, mt, :nn],
                                    ln1g[:, mt:mt + 1], ln1b[:, mt:mt + 1],
                                    op0=ALU.mult, op1=ALU.add)


def _sweep_b_tile(nc, ppsA, p3t, y1b, hbF, w1T, b1, no, nn):
    for mt in range(4):
        ps = ppsA.tile([128, 512], F32, tag="psA")
        for kt in range(2):
            nc.tensor.matmul(ps[:, :nn], w1T[:, kt, mt * 128:(mt + 1) * 128],
                             y1b[:, kt, no:no + nn], start=(kt == 0),
                             stop=(kt == 1))
        nc.scalar.activation(hbF[:, mt, no:no + nn], ps[:, :nn], ACTF.Silu,
                             bias=b1[:, mt:mt + 1])


def _sweep_c_tile(nc, d, ppsA, ppsM, p3t, y1b, hbF, w2T, b2,
                  ln2g, ln2b, ones, negone, epsc, nhalf, no, nn):
    zt = p3t.tile([128, 2, 512], BF16, tag="zt3")
    for mt in range(2):
        ps = ppsA.tile([128, 512], F32, tag="psA")
        for kt in range(4):
            nc.tensor.matmul(ps[:, :nn], w2T[:, kt, mt * 128:(mt + 1) * 128],
                             hbF[:, kt, no:no + nn], start=(kt == 0),
                             stop=(kt == 3))
        with nc.allow_low_precision(reason="LN input bf16 ok"):
            # zt = (ps + b2) + y1b  (residual + bias in one op)
            nc.vector.scalar_tensor_tensor(zt[:, mt, :nn], ps[:, :nn],
                                           b2[:, mt:mt + 1],
                                           y1b[:, mt, no:no + nn],
                                           op0=ALU.add, op1=ALU.add)
    rep = _ln_norm(nc, ppsM, p3t, zt, ones, negone, epsc, nhalf, nn,
                   tail=True)
    nrm = _norm_apply(nc, p3t, zt, rep, nn)
    for mt in range(2):
        yo = p3t.tile([128, 512], F32, tag="yo")
        nc.vector.tensor_scalar(yo[:, :nn], nrm[:, mt, :nn],
                                ln2g[:, mt:mt + 1], ln2b[:, mt:mt + 1],
                                op0=ALU.mult, op1=ALU.add)
        nc.sync.dma_start(out=d["y_out"][mt, :, no:no + nn], in_=yo[:, :nn])


BF = ml_dtypes.bfloat16
F8 = ml_dtypes.float8_e4m3


def _j68_const():
    j = np.arange(WX, dtype=np.float32)
    grid = np.broadcast_to(j[:, None], (WX, P_PTS)).reshape(1, -1)
    return np.ascontiguousarray(np.broadcast_to(grid, (128, WX * P_PTS))
                                ).astype(np.float16)


def _maskW_const():
    n = np.arange(128)
    s = np.arange(SBLK)
    m = (n[:, None] % SBLK == s[None, :]).astype(np.float16)  # [n, s]
    return np.ascontiguousarray(
        np.repeat(m[:, :, None], WY * WX, axis=2).reshape(128, -1))


def _prep_inputs(inputs):
    f = np.asarray(inputs["feats"], np.float32)
    fp = np.asarray(inputs["feats_pos"], np.float32)
    anch = np.asarray(inputs["anchor_points"], np.float32)

    def bf(x):
        return np.asarray(x, np.float32).astype(BF)

    offW = np.asarray(inputs["off_W"], np.float32)
    attnW = np.asarray(inputs["attn_W"], np.float32)
    oab = np.concatenate([np.asarray(inputs["off_b"], np.float32),
                          np.asarray(inputs["attn_b"], np.float32)])
    # value bias folded through the out projection: agg gets +vb (the hat
    # weights per group sum to ~1), so out_b_eff = out_b + vb @ out_W
    outb_eff = (np.asarray(inputs["out_b"], np.float32)
                + np.asarray(inputs["value_b"], np.float32)
                @ np.asarray(inputs["out_W"], np.float32))
    shared = {
        "vW": bf(inputs["value_W"]),
        "oaW": bf(np.concatenate([offW, attnW], axis=1)),
        "oabrow": np.ascontiguousarray(oab.reshape(1, 192)).astype(BF),
        "outW": bf(inputs["out_W"]),
        "w1T": bf(np.asarray(inputs["ffn_w1"], np.float32).T),
        "b1": np.ascontiguousarray(
            np.asarray(inputs["ffn_b1"], np.float32).reshape(4, 128).T),
        "w2T": bf(np.asarray(inputs["ffn_w2"], np.float32).T),
        "b2": np.ascontiguousarray(
            np.asarray(inputs["ffn_b2"], np.float32).reshape(2, 128).T),
        "ln1g": np.ascontiguousarray(
            np.asarray(inputs["ln1_g"], np.float32).reshape(2, 128).T),
        "ln1b": np.ascontiguousarray(
            np.asarray(inputs["ln1_b"], np.float32).reshape(2, 128).T),
        "ln2g": np.ascontiguousarray(
            np.asarray(inputs["ln2_g"], np.float32).reshape(2, 128).T),
        "ln2b": np.ascontiguousarray(
            np.asarray(inputs["ln2_b"], np.float32).reshape(2, 128).T),
        "j68": _j68_const(),
        "identh": np.eye(128, dtype=np.float32).astype(np.float16),
        "maskW": _maskW_const(),
        "ones": np.full((128, 1), 1.0 / 256, np.float32).astype(BF),
        "ones128b": np.ones((1, 128), np.float32).astype(BF),
        "negone": np.full((1, 1), -1.0, np.float32).astype(BF),
        "epsc": np.full((1, 1), LN_EPS, np.float32),
        "nhalf": np.full((1, 1), -0.5, np.float32),
    }

    qsum = (f + fp).reshape(B, C, HW).astype(np.float32)
    qbias = qsum + outb_eff[None, :, None]
    in_maps = []
    for k in range(NCORES):
        b, s = k // 4, (k % 4) * NLOC
        q8 = qsum[b].astype(F8)
        ax = anch[b].reshape(HW, 2)[s:s + NLOC, 0] * W
        ay = anch[b].reshape(HW, 2)[s:s + NLOC, 1] * H
        ox = np.clip(np.rint(ax) - WX // 2, 0, W - WX)
        oy = np.clip(np.rint(ay) - WY // 2, 0, H - WY)
        axm = (ax + 0.5 - ox).astype(np.float32)
        aym = (ay + 0.5 - oy).astype(np.float32)
        m0 = (oy * W + ox).astype(np.int64)

        # gather idx i = blk*128 + p, p = s*NROW + dy*NDXP + dxp; idx points
        # at the 512B dx-pair (2 image positions) for position blk*16+s.
        idxs = np.zeros((NCH, NBLK, 128), np.int64)           # [c, blk, p]
        pos = m0.reshape(NCH, NBLK, SBLK)                     # [c, blk, s]
        for dy in range(WY):
            for dxp in range(NDXP):
                r = dy * NDXP + dxp
                idxs[:, :, np.arange(SBLK) * NROW + r] = pos + dy * W + 2 * dxp
        # wrapped-16: idx i lives at [i % 16, i // 16], replicated x8
        flat = idxs.reshape(NCH, NBLK * 128)                  # [c, i]
        g16 = flat.reshape(NCH, 64, 16).transpose(2, 0, 1)    # [lo, c, col]
        gidx = np.tile(g16.reshape(16, NCH * 64), (8, 1)).astype(np.int16)

        m = dict(shared)
        m["qsum8"] = np.ascontiguousarray(q8.reshape(2, 128, HW))
        m["qloc8"] = np.ascontiguousarray(
            q8[:, s:s + NLOC].reshape(2, 128, NLOC))
        m["qbin"] = np.ascontiguousarray(
            qbias[b, :, s:s + NLOC].reshape(2, 128, NLOC).astype(BF))
        axy2 = np.stack([axm.reshape(NCH, 128).T,
                         aym.reshape(NCH, 128).T], axis=2)
        m["axy2"] = np.ascontiguousarray(axy2.reshape(128, NCH * 2))
        m["gidx"] = gidx
        in_maps.append(m)
    return in_maps


def kernel(**inputs):
    if "nc" not in _CACHE:
        _CACHE["nc"] = _build_program()
    nc = _CACHE["nc"]
    in_maps = _prep_inputs(inputs)
    trace = bool(int(os.environ.get("KTRACE", "0")))
    res = run_bass_kernel_spmd(nc, in_maps, core_ids=list(range(NCORES)),
                               trace=trace)
    _CACHE["exec_time_ns"] = res.exec_time_ns
    _CACHE["trace"] = res.instructions_and_trace
    out = np.zeros((B, C, HW), np.float32)
    for k in range(NCORES):
        b, s = k // 4, (k % 4) * NLOC
        out[b, :, s:s + NLOC] = res.results[k]["y_out"].reshape(C, NLOC)
    return out.reshape(B, C, H, W)
